# revision 1
# baseline (speedup 1.0000x reference)
"""BatchHardTripletLoss on 8 TRN2 NeuronCores (Bass/Tile).

Data-parallel, SPMD-uniform strategy:
  - Host: sort rows by label, pad every class segment to SEG=1024 rows with
    far-away dummy rows (first coord DUMMY_VAL -> sq ~ 1e6, so dummies never
    win a hardest-negative).  Core i gets one 128-row anchor tile per class
    (tile index (i+k)%8 of class k), so local anchor tile k is class k on
    every core and one SPMD program has fully static slicing.
  - On chip: j (candidate rows) live on PSUM partitions, anchors on the free
    dim.  TensorE computes +2*E_jtile@A^T in bf16.  One native DVE
    scalar_tensor_tensor per range fuses everything:
        acc = (psum - sq_j) max/min acc        (sq_j is per-partition!)
    hardest-negative accumulates max(2G - sq_j) over other-class tiles,
    hardest-positive accumulates min(2G - sq_j) over the own-class tile's
    real rows.  Pure-dummy j-tiles are skipped statically.
  - Both [128, NA] accumulators ship to the host, which folds the 128
    j-partials, applies sq_a, sqrt, relu(hp - hn + 1), masks dummy anchors
    and averages.  All O(B^2) work stays on hardware.
"""

import numpy as np

import concourse.bass as bass
import concourse.bacc as bacc
import concourse.tile as tile
from concourse import mybir
from concourse.bass_utils import run_bass_kernel_spmd
from concourse import bass_isa

B, D, NCLASS = 8192, 128, 10
SEG = 1024                 # padded rows per class
TPC = SEG // 128           # 128-row j-tiles per class = 8
NCORES = 8
BPAD = NCLASS * SEG        # 10240
NJT = BPAD // 128          # 80 j-tiles
NA = NCLASS * 128          # anchors per core = 1280
F32 = mybir.dt.float32
BF16 = mybir.dt.bfloat16
AFT = mybir.ActivationFunctionType
ALU = mybir.AluOpType
MARGIN = 1.0
DUMMY_VAL = 1000.0


def build_nc(R):
    """R: real row count per class (512 < R[k] <= SEG)."""
    nc = bacc.Bacc()
    ebt_d = nc.dram_tensor("ebt", [D, BPAD], F32, kind="ExternalInput")
    ant_d = nc.dram_tensor("anch_t", [D, NA], F32, kind="ExternalInput")
    epn_d = nc.dram_tensor("ep_nat", [BPAD, D], F32, kind="ExternalInput")
    out_d = nc.dram_tensor("out", [256, NA], F32, kind="ExternalOutput")
    o16_d = nc.dram_tensor("out16", [256, NA], mybir.dt.float16, kind="ExternalOutput")

    with tile.TileContext(nc) as tc:
        with (
            tc.tile_pool(name="big", bufs=1) as big,
            tc.tile_pool(name="small", bufs=1) as small,
            tc.tile_pool(name="psum", bufs=2, space=bass.MemorySpace.PSUM) as psum,
        ):
            # ---------------- load inputs ----------------
            ebt_f = big.tile([D, BPAD], F32, tag="ebt_f")
            nc.sync.dma_start(ebt_f[:], ebt_d[:])
            an_t = big.tile([D, NA], F32, tag="an_t")
            nc.sync.dma_start(an_t[:], ant_d[:])
            epn = big.tile([128, NJT, D], F32, tag="epn")
            nc.sync.dma_start(epn[:], epn_d.rearrange("(t q) d -> q t d", q=128))

            # matmul operands first so the PE can start ~11us earlier
            # (Bacc's event-semaphore pass legalizes any multi-wait fan-in)
            ebt_b = big.tile([D, BPAD], BF16, tag="ebt_b")
            nc.vector.tensor_copy(ebt_b[:], ebt_f[:])
            an2_b = big.tile([D, NA], BF16, tag="an2_b")
            nc.vector.tensor_scalar_mul(an2_b[:], an_t[:], 2.0)

            # sqv[q, t] = ||e_{t*128+q}||^2
            nc.scalar.activation(epn[:], epn[:], AFT.Square)
            sqv = small.tile([128, NJT], F32, tag="sqv")
            nc.vector.reduce_sum(sqv[:], epn[:], axis=mybir.AxisListType.X)
            negsqv = small.tile([128, NJT], F32, tag="negsqv")
            nc.vector.tensor_scalar_mul(negsqv[:], sqv[:], -1.0)

            # accumulators: no memset -- the first touch of each region is a
            # plain write (keeps every DVE op at <=1 sync wait).  Two paths:
            # f32 accs fed by DVE STT straight off PSUM, fp16 accs fed by an
            # ACT bias-add pass (psum - sq_j) then 2x-mode fp16 DVE TT.
            acc_hn = big.tile([128, NA], F32, tag="acc_hn")
            acc_hp = big.tile([128, NA], F32, tag="acc_hp")
            F16 = mybir.dt.float16
            acc_hn16 = big.tile([128, NA], F16, tag="acc_hn16")
            acc_hp16 = big.tile([128, NA], F16, tag="acc_hp16")

            # ---------------- main loop over 80 j-tiles ----------------
            touched_hn, touched_hp = set(), set()
            touched_hn16, touched_hp16 = set(), set()
            for t in range(NJT):
                c, ri = t // TPC, t % TPC
                nreal = min(max(int(R[c]) - ri * 128, 0), 128)
                if nreal == 0:
                    # pure padding rows: can never win a hardest-neg/pos
                    continue
                g = psum.tile([128, NA], F32, tag="ps")
                for h, w in ((0, 512), (512, 512), (1024, 256)):
                    nc.tensor.matmul(g[:, h:h + w],
                                     ebt_b[:, t * 128:(t + 1) * 128],
                                     an2_b[:, h:h + w], start=True, stop=True)
                sq_t = sqv[:, t:t + 1]
                act_path = (t % 4) != 0
                if act_path:
                    tmp16 = small.tile([128, NA], mybir.dt.float16, tag="tmp16",
                                       bufs=4)
                    nc.scalar.add(tmp16[:], g[:], negsqv[:, t:t + 1])

                def upd(acc, rows, lo, hi, op1, touched, key):
                    seg, segs = None, []
                    for k in range(lo // 128, hi // 128):
                        if (key, k) in touched:
                            if seg and seg[2]:
                                seg = (seg[0], k + 1, True)
                                segs[-1] = seg
                            elif seg is None or not seg[2]:
                                seg = (k, k + 1, True)
                                segs.append(seg)
                            else:
                                seg = (k, k + 1, True)
                                segs.append(seg)
                        else:
                            if seg and not seg[2]:
                                seg = (seg[0], k + 1, False)
                                segs[-1] = seg
                            else:
                                seg = (k, k + 1, False)
                                segs.append(seg)
                            touched.add((key, k))
                    for a, b, is_acc in segs:
                        sl = slice(a * 128, b * 128)
                        if act_path:
                            if is_acc:
                                nc.vector.tensor_tensor(
                                    acc[rows, sl], acc[rows, sl],
                                    tmp16[rows, sl], op=op1)
                            else:
                                nc.vector.tensor_copy(acc[rows, sl],
                                                      tmp16[rows, sl])
                        elif is_acc:
                            nc.vector.scalar_tensor_tensor(
                                acc[rows, sl], g[rows, sl], sq_t[rows],
                                acc[rows, sl], op0=ALU.subtract, op1=op1)
                        else:
                            nc.vector.tensor_scalar(
                                acc[rows, sl], g[rows, sl], sq_t[rows], None,
                                op0=ALU.subtract)

                a_hn = acc_hn16 if act_path else acc_hn
                a_hp = acc_hp16 if act_path else acc_hp
                t_hn = touched_hn16 if act_path else touched_hn
                t_hp = touched_hp16 if act_path else touched_hp
                # hardest-negative: all anchor columns except own class c
                if c > 0:
                    upd(a_hn, slice(0, 128), 0, c * 128, ALU.max, t_hn, 0)
                if c < NCLASS - 1:
                    upd(a_hn, slice(0, 128), (c + 1) * 128, NA, ALU.max, t_hn, 0)
                # hardest-positive: own-class columns, real j rows only
                if nreal > 0:
                    upd(a_hp, slice(0, nreal), c * 128, (c + 1) * 128, ALU.min,
                        t_hp, 0)

            # ---------------- ship all accumulators; host folds ----------
            nc.sync.dma_start(out_d[0:128, :], acc_hn[:])
            nc.sync.dma_start(out_d[128:256, :], acc_hp[:])
            nc.sync.dma_start(o16_d[0:128, :], acc_hn16[:])
            nc.sync.dma_start(o16_d[128:256, :], acc_hp16[:])
    nc.compile()
    return nc


def prepare(embeddings, labels):
    emb = np.ascontiguousarray(np.asarray(embeddings, dtype=np.float32))
    lab = np.asarray(labels).astype(np.int64).ravel()
    assert emb.shape == (B, D)
    order = np.argsort(lab, kind="stable")
    es = emb[order]
    counts = np.bincount(lab, minlength=NCLASS)
    assert counts.max() <= SEG, counts
    ep = np.zeros((BPAD, D), np.float32)
    ep[:, 0] = DUMMY_VAL
    ofs = np.concatenate([[0], np.cumsum(counts)])
    for c in range(NCLASS):
        ep[c * SEG: c * SEG + counts[c]] = es[ofs[c]:ofs[c + 1]]
    ebt = np.ascontiguousarray(ep.T)
    in_maps = []
    for i in range(NCORES):
        rows = [
            ep[k * SEG + ((i + k) % TPC) * 128: k * SEG + ((i + k) % TPC + 1) * 128]
            for k in range(NCLASS)
        ]
        anch = np.ascontiguousarray(np.concatenate(rows, 0))
        in_maps.append({
            "ebt": ebt,
            "anch_t": np.ascontiguousarray(anch.T),
            "ep_nat": ep,
        })
    return in_maps, counts


def combine(results, counts, in_maps):
    total = 0.0
    for i in range(NCORES):
        o = np.asarray(results[i]["out"], np.float32)
        o16 = np.asarray(results[i]["out16"], np.float32)
        hn_m = np.maximum(np.max(o[0:128], axis=0), np.max(o16[0:128], axis=0))
        hp_m = np.minimum(np.min(o[128:256], axis=0), np.min(o16[128:256], axis=0))
        anch = in_maps[i]["anch_t"].T.astype(np.float32)
        sq_a = np.sum(anch * anch, axis=1)
        hn = np.sqrt(np.maximum(sq_a - hn_m, 0.0))
        hp = np.sqrt(np.maximum(sq_a - hp_m, 0.0))
        li = np.maximum(hp - hn + np.float32(MARGIN), 0.0)
        for k in range(NCLASS):
            t = (i + k) % TPC
            nreal = int(np.clip(int(counts[k]) - t * 128, 0, 128))
            if nreal > 0:
                total += float(np.sum(li[k * 128: k * 128 + nreal].astype(np.float64)))
    return np.asarray(total / B, dtype=np.float32)


def kernel(embeddings, labels, _trace=False, _tmpdir=None):
    in_maps, counts = prepare(embeddings, labels)
    nc = build_nc(list(counts))
    res = run_bass_kernel_spmd(
        nc, in_maps, list(range(NCORES)), trace=_trace, tmpdir=_tmpdir
    )
    out = combine(res.results, counts, in_maps)
    if _trace:
        return out, res
    return out



# revision 2
# speedup vs baseline: 7.9376x; 7.9376x over previous
"""BatchHardTripletLoss on 8 TRN2 NeuronCores (Bass/Tile).

The warm-path cost of this problem is host<->device traffic over the axon
tunnel (~60 MB/s up, ~30 MB/s down), not on-chip compute, so the kernel is
built around moving as few bytes as possible:

  - Host: sort rows by label, pad every class segment to SEG=1024 rows with
    far-away dummy rows (first coord DUMMY_VAL -> sq ~ 1e6, so dummies never
    win a hardest-negative).  Core i uploads ONLY its anchor shard: tile i of
    every class, transposed, in fp16 ([128, 1280] = 327 KB), plus a 5 KB
    anchor-validity mask.  Total upload ~2.7 MB instead of ~105 MB.
  - On chip: an AllGather over NeuronLink reassembles the full padded
    embedding matrix (the 8 anchor shards tile it exactly).  Squared norms
    are computed on chip (square + ones-matmul partition reduction), the
    distance-matrix sweep accumulates hardest-pos/neg in "2*dot - sq_j"
    space via DVE scalar_tensor_tensor off PSUM, and a PE-transpose fold
    reduces everything to one f32 loss partial per core ([1,1] download).
  - Host folds 8 scalars.
"""

import numpy as np

import concourse.bass as bass
import concourse.bacc as bacc
import concourse.tile as tile
from concourse import masks, mybir
from concourse.bass_utils import run_bass_kernel_spmd

B, D, NCLASS = 8192, 128, 10
SEG = 1024                 # padded rows per class
TPC = SEG // 128           # 128-row tiles per class = 8
NCORES = 8
BPAD = NCLASS * SEG        # 10240
NJT = BPAD // 128          # 80 j-tiles
NA = NCLASS * 128          # anchors per core = 1280
F32 = mybir.dt.float32
F16 = mybir.dt.float16
AFT = mybir.ActivationFunctionType
ALU = mybir.AluOpType
MARGIN = 1.0
DUMMY_VAL = 1000.0


def build_nc(R):
    """R: real row count per class (0 <= R[k] <= SEG)."""
    nc = bacc.Bacc()
    esh_d = nc.dram_tensor("esh", [D, NA], F16, kind="ExternalInput")
    amask_d = nc.dram_tensor("amask", [128, NCLASS], F32, kind="ExternalInput")
    loss_d = nc.dram_tensor("loss", [1, 1], F32, kind="ExternalOutput")

    with tile.TileContext(nc) as tc:
        with (
            tc.tile_pool(name="sb", bufs=1) as sb,
            tc.tile_pool(name="dram", bufs=1, space="DRAM") as dram,
        ):
            # ---- kick off the AllGather first: cores exchange anchor shards
            # so each one can rebuild the full [128, 10240] embedding matrix.
            bounce = dram.tile([D, NA], F16, tag="bounce")
            gath = dram.tile([NCORES * D, NA], F16, tag="gath")
            nc.gpsimd.dma_start(bounce[:], esh_d[:])
            nc.gpsimd.collective_compute(
                "AllGather",
                ALU.bypass,
                replica_groups=[list(range(NCORES))],
                ins=[bounce.opt()],
                outs=[gath.opt()],
            )

            # ---- local anchor-side prep (overlaps the collective)
            esh_sb = sb.tile([D, NA], F16, tag="esh_sb")
            nc.sync.dma_start(esh_sb[:], esh_d[:])
            amask_sb = sb.tile([128, NCLASS], F32, tag="amask_sb")
            nc.sync.dma_start(amask_sb[:], amask_d[:])

            an2 = sb.tile([D, NA], F16, tag="an2")
            nc.vector.tensor_scalar_mul(an2[:], esh_sb[:], 2.0)

            ones = sb.tile([128, 1], F32, tag="ones")
            nc.vector.memset(ones[:], 1.0)

            # squared norms: square, then partition-reduce via ones-matmul
            sqaf = sb.tile([128, NA], F32, tag="sqaf")
            nc.scalar.activation(sqaf[:], esh_sb[:], AFT.Square)
            sqa_row = sb.tile([1, NA], F32, tag="sqa_row")
            sq_row = sb.tile([1, BPAD], F32, tag="sq_row")
            eb = sb.tile([128, NCLASS, TPC, 128], F16, tag="eb")
            sqf = sb.tile([128, NCLASS, TPC, 128], F32, tag="sqf")
            with tc.tile_pool(name="ps_sq", bufs=2, space=bass.MemorySpace.PSUM) as ps_sq:
                for h, w in ((0, 512), (512, 512), (1024, 256)):
                    pt = ps_sq.tile([1, 512], F32, tag="pt")
                    nc.tensor.matmul(pt[0:1, 0:w], ones[:], sqaf[:, h:h + w],
                                     start=True, stop=True)
                    nc.scalar.copy(sqa_row[0:1, h:h + w], pt[0:1, 0:w])

                # full matrix: gath[(t d), (k q)] -> eb[d, k, t, q]
                # (core t's shard holds tile t of every class k)
                nc.sync.dma_start(
                    eb[:], gath[:].rearrange("(t d) (k q) -> d k t q", d=128, k=NCLASS)
                )
                nc.scalar.activation(sqf[:], eb[:], AFT.Square)
                for k in range(NCLASS):
                    for t0 in (0, 4):
                        pt = ps_sq.tile([1, 512], F32, tag="pt")
                        nc.tensor.matmul(pt[0:1, :], ones[:], sqf[:, k, t0:t0 + 4, :],
                                         start=True, stop=True)
                        nc.scalar.copy(
                            sq_row[0:1, k * SEG + t0 * 128: k * SEG + t0 * 128 + 512],
                            pt[0:1, :])

            # reshape the [1, N] rows to per-partition layout via a DRAM bounce
            scr_a = dram.tile([1, NA], F32, tag="scr_a")
            scr_j = dram.tile([1, BPAD], F32, tag="scr_j")
            nc.sync.dma_start(scr_a[:], sqa_row[:])
            nc.sync.dma_start(scr_j[:], sq_row[:])
            sqa_pk = sb.tile([128, NCLASS], F32, tag="sqa_pk")
            nc.sync.dma_start(sqa_pk[:], scr_a[:].rearrange("a (k q) -> (a q) k", q=128))
            sqv = sb.tile([128, NJT], F32, tag="sqv")
            nc.sync.dma_start(sqv[:], scr_j[:].rearrange("a (t q) -> (a q) t", q=128))

            # ---- main sweep over 80 j-tiles, accumulating in 2*dot - sq_j space
            acc_hn = sb.tile([128, NA], F32, tag="acc_hn")
            acc_hp = sb.tile([128, NA], F32, tag="acc_hp")
            nc.vector.memset(acc_hn[:], -3.0e38)
            nc.vector.memset(acc_hp[:], 3.0e38)

            with tc.tile_pool(name="ps_g", bufs=2, space=bass.MemorySpace.PSUM) as ps_g:
                for t in range(NJT):
                    c, ri = t // TPC, t % TPC
                    nreal = min(max(int(R[c]) - ri * 128, 0), 128)
                    if nreal == 0:
                        # pure padding rows: can never win a hardest-neg/pos
                        continue
                    g = ps_g.tile([128, NA], F32, tag="g")
                    for h, w in ((0, 512), (512, 512), (1024, 256)):
                        nc.tensor.matmul(g[:, h:h + w], eb[:, c, ri, :],
                                         an2[:, h:h + w], start=True, stop=True)
                    sq_t = sqv[:, t:t + 1]
                    # hardest-negative: all anchor columns except own class c
                    if c > 0:
                        nc.vector.scalar_tensor_tensor(
                            acc_hn[:, 0:c * 128], g[:, 0:c * 128], sq_t,
                            acc_hn[:, 0:c * 128], op0=ALU.subtract, op1=ALU.max)
                    if c < NCLASS - 1:
                        nc.vector.scalar_tensor_tensor(
                            acc_hn[:, (c + 1) * 128:NA], g[:, (c + 1) * 128:NA], sq_t,
                            acc_hn[:, (c + 1) * 128:NA], op0=ALU.subtract, op1=ALU.max)
                    # hardest-positive: own-class columns, real j rows only
                    sl = slice(c * 128, (c + 1) * 128)
                    nc.vector.scalar_tensor_tensor(
                        acc_hp[0:nreal, sl], g[0:nreal, sl], sqv[0:nreal, t:t + 1],
                        acc_hp[0:nreal, sl], op0=ALU.subtract, op1=ALU.min)

            # ---- fold on chip: transpose-reduce over j-partials, loss math,
            # and a final partition sum down to [1, 1]
            ident = sb.tile([128, 128], F32, tag="ident")
            masks.make_identity(nc, ident[:])
            hn_t = sb.tile([128, NCLASS], F32, tag="hn_t")
            hp_t = sb.tile([128, NCLASS], F32, tag="hp_t")
            with tc.tile_pool(name="ps_f", bufs=2, space=bass.MemorySpace.PSUM) as ps_f:
                for b in range(NCLASS):
                    pn = ps_f.tile([128, 128], F32, tag="pn")
                    nc.tensor.transpose(pn[:], acc_hn[:, b * 128:(b + 1) * 128], ident[:])
                    nc.vector.reduce_max(hn_t[:, b:b + 1], pn[:], axis=mybir.AxisListType.X)
                    pp = ps_f.tile([128, 128], F32, tag="pp")
                    nc.tensor.transpose(pp[:], acc_hp[:, b * 128:(b + 1) * 128], ident[:])
                    nc.vector.tensor_reduce(hp_t[:, b:b + 1], pp[:], op=ALU.min,
                                            axis=mybir.AxisListType.X)

                hn2 = sb.tile([128, NCLASS], F32, tag="hn2")
                nc.vector.tensor_tensor(hn2[:], sqa_pk[:], hn_t[:], op=ALU.subtract)
                nc.vector.tensor_scalar_max(hn2[:], hn2[:], 0.0)
                nc.scalar.sqrt(hn2[:], hn2[:])
                hp2 = sb.tile([128, NCLASS], F32, tag="hp2")
                nc.vector.tensor_tensor(hp2[:], sqa_pk[:], hp_t[:], op=ALU.subtract)
                nc.vector.tensor_scalar_max(hp2[:], hp2[:], 0.0)
                nc.scalar.sqrt(hp2[:], hp2[:])

                li = sb.tile([128, NCLASS], F32, tag="li")
                nc.vector.tensor_tensor(li[:], hp2[:], hn2[:], op=ALU.subtract)
                nc.vector.tensor_scalar(li[:], li[:], float(MARGIN), 0.0,
                                        op0=ALU.add, op1=ALU.max)
                nc.vector.tensor_tensor(li[:], li[:], amask_sb[:], op=ALU.mult)
                li1 = sb.tile([128, 1], F32, tag="li1")
                nc.vector.reduce_sum(li1[:], li[:], axis=mybir.AxisListType.X)
                pl = ps_f.tile([1, 1], F32, tag="pl")
                nc.tensor.matmul(pl[0:1, 0:1], li1[:], ones[:], start=True, stop=True)
                loss_sb = sb.tile([1, 1], F32, tag="loss_sb")
                nc.scalar.copy(loss_sb[:], pl[0:1, 0:1])
            nc.sync.dma_start(loss_d[:], loss_sb[:])
    nc.compile()
    return nc


def prepare(embeddings, labels):
    emb = np.ascontiguousarray(np.asarray(embeddings, dtype=np.float32))
    lab = np.asarray(labels).astype(np.int64).ravel()
    assert emb.shape == (B, D)
    order = np.argsort(lab, kind="stable")
    es = emb[order]
    counts = np.bincount(lab, minlength=NCLASS)
    assert counts.max() <= SEG, counts
    ep = np.zeros((BPAD, D), np.float32)
    ep[:, 0] = DUMMY_VAL
    ofs = np.concatenate([[0], np.cumsum(counts)])
    for c in range(NCLASS):
        ep[c * SEG: c * SEG + counts[c]] = es[ofs[c]:ofs[c + 1]]
    eph = ep.astype(np.float16)
    q = np.arange(128)
    in_maps = []
    for i in range(NCORES):
        rows = np.concatenate(
            [eph[k * SEG + i * 128: k * SEG + (i + 1) * 128] for k in range(NCLASS)], 0
        )
        esh = np.ascontiguousarray(rows.T)                       # [128, 1280] f16
        amask = (i * 128 + q[:, None] < counts[None, :]).astype(np.float32)
        in_maps.append({"esh": esh, "amask": amask})
    return in_maps, counts


def combine(results, counts=None, in_maps=None):
    total = 0.0
    for i in range(NCORES):
        total += float(np.asarray(results[i]["loss"], np.float32)[0, 0])
    return np.asarray(total / B, dtype=np.float32)


def kernel(embeddings, labels, _trace=False, _tmpdir=None):
    in_maps, counts = prepare(embeddings, labels)
    nc = build_nc(list(counts))
    res = run_bass_kernel_spmd(
        nc, in_maps, list(range(NCORES)), trace=_trace, tmpdir=_tmpdir
    )
    out = combine(res.results)
    if _trace:
        return out, res
    return out


# revision 3
# speedup vs baseline: 14.7618x; 1.8597x over previous
"""BatchHardTripletLoss on 8 TRN2 NeuronCores (Bass/Tile).

The warm-path cost of this problem is host<->device traffic over the axon
tunnel (~60 MB/s up, ~30 MB/s down), not on-chip compute, so the kernel is
built around moving as few bytes as possible:

  - Host: sort rows by label, pad every class segment to SEG=1024 rows with
    far-away dummy rows (first coord DUMMY_VAL -> sq ~ 1e6, so dummies never
    win a hardest-negative).  Core i uploads ONLY its anchor shard: tile i of
    every class, transposed, in fp16 ([128, 1280] = 327 KB), plus a 5 KB
    anchor-validity mask.  Total upload ~2.7 MB instead of ~105 MB.
  - On chip: an AllGather over NeuronLink reassembles the full padded
    embedding matrix (the 8 anchor shards tile it exactly).  Squared norms
    are computed on chip (square + ones-matmul partition reduction), the
    distance-matrix sweep accumulates hardest-pos/neg in "2*dot - sq_j"
    space via DVE scalar_tensor_tensor off PSUM, and a PE-transpose fold
    reduces everything to one f32 loss partial per core ([1,1] download).
  - Host folds 8 scalars.
"""

import numpy as np

import jax

# Cache compiled XLA executables on disk: run_bass_kernel_spmd builds a fresh
# jax.jit closure per call, so without this every warm call pays a ~165 ms
# recompile before dispatch.
jax.config.update("jax_compilation_cache_dir", "/tmp/jax_comp_cache")
jax.config.update("jax_persistent_cache_min_compile_time_secs", 0.0)
jax.config.update("jax_persistent_cache_min_entry_size_bytes", 0)

import concourse.bass as bass
import concourse.bacc as bacc
import concourse.tile as tile
from concourse import masks, mybir
from concourse.bass_utils import run_bass_kernel_spmd

B, D, NCLASS = 8192, 128, 10
SEG = 1024                 # padded rows per class
TPC = SEG // 128           # 128-row tiles per class = 8
NCORES = 8
BPAD = NCLASS * SEG        # 10240
NJT = BPAD // 128          # 80 j-tiles
NA = NCLASS * 128          # anchors per core = 1280
F32 = mybir.dt.float32
F16 = mybir.dt.float16
AFT = mybir.ActivationFunctionType
ALU = mybir.AluOpType
MARGIN = 1.0
DUMMY_VAL = 1000.0


def build_nc(R):
    """R: real row count per class (0 <= R[k] <= SEG)."""
    nc = bacc.Bacc()
    esh_d = nc.dram_tensor("esh", [D, NA], F16, kind="ExternalInput")
    amask_d = nc.dram_tensor("amask", [128, NCLASS], F32, kind="ExternalInput")
    loss_d = nc.dram_tensor("loss", [1, 1], F32, kind="ExternalOutput")

    with tile.TileContext(nc) as tc:
        with (
            tc.tile_pool(name="sb", bufs=1) as sb,
            tc.tile_pool(name="dram", bufs=1, space="DRAM") as dram,
        ):
            # ---- kick off the AllGather first: cores exchange anchor shards
            # so each one can rebuild the full [128, 10240] embedding matrix.
            bounce = dram.tile([D, NA], F16, tag="bounce")
            gath = dram.tile([NCORES * D, NA], F16, tag="gath")
            nc.gpsimd.dma_start(bounce[:], esh_d[:])
            nc.gpsimd.collective_compute(
                "AllGather",
                ALU.bypass,
                replica_groups=[list(range(NCORES))],
                ins=[bounce.opt()],
                outs=[gath.opt()],
            )

            # ---- local anchor-side prep (overlaps the collective)
            esh_sb = sb.tile([D, NA], F16, tag="esh_sb")
            nc.sync.dma_start(esh_sb[:], esh_d[:])
            amask_sb = sb.tile([128, NCLASS], F32, tag="amask_sb")
            nc.sync.dma_start(amask_sb[:], amask_d[:])

            an2 = sb.tile([D, NA], F16, tag="an2")
            nc.vector.tensor_scalar_mul(an2[:], esh_sb[:], 2.0)

            ones = sb.tile([128, 1], F32, tag="ones")
            nc.vector.memset(ones[:], 1.0)

            # squared norms: square, then partition-reduce via ones-matmul
            sqaf = sb.tile([128, NA], F32, tag="sqaf")
            nc.scalar.activation(sqaf[:], esh_sb[:], AFT.Square)
            sqa_row = sb.tile([1, NA], F32, tag="sqa_row")
            sq_row = sb.tile([1, BPAD], F32, tag="sq_row")
            eb = sb.tile([128, NCLASS, TPC, 128], F16, tag="eb")
            sqf = sb.tile([128, NCLASS, TPC, 128], F32, tag="sqf")
            with tc.tile_pool(name="ps_sq", bufs=2, space=bass.MemorySpace.PSUM) as ps_sq:
                for h, w in ((0, 512), (512, 512), (1024, 256)):
                    pt = ps_sq.tile([1, 512], F32, tag="pt")
                    nc.tensor.matmul(pt[0:1, 0:w], ones[:], sqaf[:, h:h + w],
                                     start=True, stop=True)
                    nc.scalar.copy(sqa_row[0:1, h:h + w], pt[0:1, 0:w])

                # full matrix: gath[(t d), (k q)] -> eb[d, k, t, q]
                # (core t's shard holds tile t of every class k)
                nc.sync.dma_start(
                    eb[:], gath[:].rearrange("(t d) (k q) -> d k t q", d=128, k=NCLASS)
                )
                nc.scalar.activation(sqf[:], eb[:], AFT.Square)
                for k in range(NCLASS):
                    for t0 in (0, 4):
                        pt = ps_sq.tile([1, 512], F32, tag="pt")
                        nc.tensor.matmul(pt[0:1, :], ones[:], sqf[:, k, t0:t0 + 4, :],
                                         start=True, stop=True)
                        nc.scalar.copy(
                            sq_row[0:1, k * SEG + t0 * 128: k * SEG + t0 * 128 + 512],
                            pt[0:1, :])

            # reshape the [1, N] rows to per-partition layout via a DRAM bounce
            scr_a = dram.tile([1, NA], F32, tag="scr_a")
            scr_j = dram.tile([1, BPAD], F32, tag="scr_j")
            nc.sync.dma_start(scr_a[:], sqa_row[:])
            nc.sync.dma_start(scr_j[:], sq_row[:])
            sqa_pk = sb.tile([128, NCLASS], F32, tag="sqa_pk")
            nc.sync.dma_start(sqa_pk[:], scr_a[:].rearrange("a (k q) -> (a q) k", q=128))
            sqv = sb.tile([128, NJT], F32, tag="sqv")
            nc.sync.dma_start(sqv[:], scr_j[:].rearrange("a (t q) -> (a q) t", q=128))

            # ---- main sweep over 80 j-tiles, accumulating in 2*dot - sq_j space
            acc_hn = sb.tile([128, NA], F32, tag="acc_hn")
            acc_hp = sb.tile([128, NA], F32, tag="acc_hp")
            nc.vector.memset(acc_hn[:], -3.0e38)
            nc.vector.memset(acc_hp[:], 3.0e38)

            with tc.tile_pool(name="ps_g", bufs=2, space=bass.MemorySpace.PSUM) as ps_g:
                for t in range(NJT):
                    c, ri = t // TPC, t % TPC
                    nreal = min(max(int(R[c]) - ri * 128, 0), 128)
                    if nreal == 0:
                        # pure padding rows: can never win a hardest-neg/pos
                        continue
                    g = ps_g.tile([128, NA], F32, tag="g")
                    for h, w in ((0, 512), (512, 512), (1024, 256)):
                        nc.tensor.matmul(g[:, h:h + w], eb[:, c, ri, :],
                                         an2[:, h:h + w], start=True, stop=True)
                    sq_t = sqv[:, t:t + 1]
                    # hardest-negative: all anchor columns except own class c
                    if c > 0:
                        nc.vector.scalar_tensor_tensor(
                            acc_hn[:, 0:c * 128], g[:, 0:c * 128], sq_t,
                            acc_hn[:, 0:c * 128], op0=ALU.subtract, op1=ALU.max)
                    if c < NCLASS - 1:
                        nc.vector.scalar_tensor_tensor(
                            acc_hn[:, (c + 1) * 128:NA], g[:, (c + 1) * 128:NA], sq_t,
                            acc_hn[:, (c + 1) * 128:NA], op0=ALU.subtract, op1=ALU.max)
                    # hardest-positive: own-class columns, real j rows only
                    sl = slice(c * 128, (c + 1) * 128)
                    nc.vector.scalar_tensor_tensor(
                        acc_hp[0:nreal, sl], g[0:nreal, sl], sqv[0:nreal, t:t + 1],
                        acc_hp[0:nreal, sl], op0=ALU.subtract, op1=ALU.min)

            # ---- fold on chip: transpose-reduce over j-partials, loss math,
            # and a final partition sum down to [1, 1]
            ident = sb.tile([128, 128], F32, tag="ident")
            masks.make_identity(nc, ident[:])
            hn_t = sb.tile([128, NCLASS], F32, tag="hn_t")
            hp_t = sb.tile([128, NCLASS], F32, tag="hp_t")
            with tc.tile_pool(name="ps_f", bufs=2, space=bass.MemorySpace.PSUM) as ps_f:
                for b in range(NCLASS):
                    pn = ps_f.tile([128, 128], F32, tag="pn")
                    nc.tensor.transpose(pn[:], acc_hn[:, b * 128:(b + 1) * 128], ident[:])
                    nc.vector.reduce_max(hn_t[:, b:b + 1], pn[:], axis=mybir.AxisListType.X)
                    pp = ps_f.tile([128, 128], F32, tag="pp")
                    nc.tensor.transpose(pp[:], acc_hp[:, b * 128:(b + 1) * 128], ident[:])
                    nc.vector.tensor_reduce(hp_t[:, b:b + 1], pp[:], op=ALU.min,
                                            axis=mybir.AxisListType.X)

                hn2 = sb.tile([128, NCLASS], F32, tag="hn2")
                nc.vector.tensor_tensor(hn2[:], sqa_pk[:], hn_t[:], op=ALU.subtract)
                nc.vector.tensor_scalar_max(hn2[:], hn2[:], 0.0)
                nc.scalar.sqrt(hn2[:], hn2[:])
                hp2 = sb.tile([128, NCLASS], F32, tag="hp2")
                nc.vector.tensor_tensor(hp2[:], sqa_pk[:], hp_t[:], op=ALU.subtract)
                nc.vector.tensor_scalar_max(hp2[:], hp2[:], 0.0)
                nc.scalar.sqrt(hp2[:], hp2[:])

                li = sb.tile([128, NCLASS], F32, tag="li")
                nc.vector.tensor_tensor(li[:], hp2[:], hn2[:], op=ALU.subtract)
                nc.vector.tensor_scalar(li[:], li[:], float(MARGIN), 0.0,
                                        op0=ALU.add, op1=ALU.max)
                nc.vector.tensor_tensor(li[:], li[:], amask_sb[:], op=ALU.mult)
                li1 = sb.tile([128, 1], F32, tag="li1")
                nc.vector.reduce_sum(li1[:], li[:], axis=mybir.AxisListType.X)
                pl = ps_f.tile([1, 1], F32, tag="pl")
                nc.tensor.matmul(pl[0:1, 0:1], li1[:], ones[:], start=True, stop=True)
                loss_sb = sb.tile([1, 1], F32, tag="loss_sb")
                nc.scalar.copy(loss_sb[:], pl[0:1, 0:1])
            nc.sync.dma_start(loss_d[:], loss_sb[:])
    nc.compile()
    return nc


def prepare(embeddings, labels):
    emb = np.ascontiguousarray(np.asarray(embeddings, dtype=np.float32))
    lab = np.asarray(labels).astype(np.int64).ravel()
    assert emb.shape == (B, D)
    order = np.argsort(lab, kind="stable")
    es = emb[order]
    counts = np.bincount(lab, minlength=NCLASS)
    assert counts.max() <= SEG, counts
    ep = np.zeros((BPAD, D), np.float32)
    ep[:, 0] = DUMMY_VAL
    ofs = np.concatenate([[0], np.cumsum(counts)])
    for c in range(NCLASS):
        ep[c * SEG: c * SEG + counts[c]] = es[ofs[c]:ofs[c + 1]]
    eph = ep.astype(np.float16)
    q = np.arange(128)
    in_maps = []
    for i in range(NCORES):
        rows = np.concatenate(
            [eph[k * SEG + i * 128: k * SEG + (i + 1) * 128] for k in range(NCLASS)], 0
        )
        esh = np.ascontiguousarray(rows.T)                       # [128, 1280] f16
        amask = (i * 128 + q[:, None] < counts[None, :]).astype(np.float32)
        in_maps.append({"esh": esh, "amask": amask})
    return in_maps, counts


def combine(results, counts=None, in_maps=None):
    total = 0.0
    for i in range(NCORES):
        total += float(np.asarray(results[i]["loss"], np.float32)[0, 0])
    return np.asarray(total / B, dtype=np.float32)


def kernel(embeddings, labels, _trace=False, _tmpdir=None):
    in_maps, counts = prepare(embeddings, labels)
    nc = build_nc(list(counts))
    res = run_bass_kernel_spmd(
        nc, in_maps, list(range(NCORES)), trace=_trace, tmpdir=_tmpdir
    )
    out = combine(res.results)
    if _trace:
        return out, res
    return out


# revision 4
# speedup vs baseline: 16.5250x; 1.1194x over previous
"""BatchHardTripletLoss on 8 TRN2 NeuronCores (Bass/Tile).

The warm-path cost of this problem is host<->device traffic over the axon
tunnel (~60 MB/s up, ~30 MB/s down), not on-chip compute, so the kernel is
built around moving as few bytes as possible:

  - Host: sort rows by label, pad every class segment to SEG=1024 rows with
    far-away dummy rows (first coord DUMMY_VAL -> sq ~ 1e6, so dummies never
    win a hardest-negative).  Core i uploads ONLY its anchor shard: tile i of
    every class, transposed, in fp16 ([128, 1280] = 327 KB), plus a 5 KB
    anchor-validity mask.  Total upload ~2.7 MB instead of ~105 MB.
  - On chip: an AllGather over NeuronLink reassembles the full padded
    embedding matrix (the 8 anchor shards tile it exactly).  Squared norms
    are computed on chip (square + ones-matmul partition reduction), the
    distance-matrix sweep accumulates hardest-pos/neg in "2*dot - sq_j"
    space via DVE scalar_tensor_tensor off PSUM, and a PE-transpose fold
    reduces everything to one f32 loss partial per core ([1,1] download).
  - Host folds 8 scalars.
"""

import numpy as np

import jax

# Cache compiled XLA executables on disk: run_bass_kernel_spmd builds a fresh
# jax.jit closure per call, so without this every warm call pays a ~165 ms
# recompile before dispatch.
try:
    jax.config.update("jax_compilation_cache_dir", "/tmp/jax_comp_cache")
    jax.config.update("jax_persistent_cache_min_compile_time_secs", 0.0)
    jax.config.update("jax_persistent_cache_min_entry_size_bytes", 0)
except Exception:
    pass

import concourse.bass as bass
import concourse.bacc as bacc
import concourse.tile as tile
from concourse import masks, mybir
from concourse.bass_utils import run_bass_kernel_spmd

B, D, NCLASS = 8192, 128, 10
SEG = 1024                 # padded rows per class
TPC = SEG // 128           # 128-row tiles per class = 8
NCORES = 8
BPAD = NCLASS * SEG        # 10240
NJT = BPAD // 128          # 80 j-tiles
NA = NCLASS * 128          # anchors per core = 1280
F32 = mybir.dt.float32
F16 = mybir.dt.float16
AFT = mybir.ActivationFunctionType
ALU = mybir.AluOpType
MARGIN = 1.0
DUMMY_VAL = 1000.0


def build_nc(R):
    """R: real row count per class (0 <= R[k] <= SEG)."""
    nc = bacc.Bacc()
    esh_d = nc.dram_tensor("esh", [D, NA], F16, kind="ExternalInput")
    amask_d = nc.dram_tensor("amask", [128, NCLASS], F32, kind="ExternalInput")
    loss_d = nc.dram_tensor("loss", [1, 1], F32, kind="ExternalOutput")

    with tile.TileContext(nc) as tc:
        with (
            tc.tile_pool(name="sb", bufs=1) as sb,
            tc.tile_pool(name="dram", bufs=1, space="DRAM") as dram,
        ):
            # ---- kick off the AllGather first: cores exchange anchor shards
            # so each one can rebuild the full [128, 10240] embedding matrix.
            bounce = dram.tile([D, NA], F16, tag="bounce")
            gath = dram.tile([NCORES * D, NA], F16, tag="gath")
            nc.gpsimd.dma_start(bounce[:], esh_d[:])
            nc.gpsimd.collective_compute(
                "AllGather",
                ALU.bypass,
                replica_groups=[list(range(NCORES))],
                ins=[bounce.opt()],
                outs=[gath.opt()],
            )

            # ---- local anchor-side prep (overlaps the collective)
            esh_sb = sb.tile([D, NA], F16, tag="esh_sb")
            nc.sync.dma_start(esh_sb[:], esh_d[:])
            amask_sb = sb.tile([128, NCLASS], F32, tag="amask_sb")
            nc.sync.dma_start(amask_sb[:], amask_d[:])

            an2 = sb.tile([D, NA], F16, tag="an2")
            nc.vector.tensor_scalar_mul(an2[:], esh_sb[:], 2.0)

            ones = sb.tile([128, 1], F32, tag="ones")
            nc.vector.memset(ones[:], 1.0)

            # squared norms: square, then partition-reduce via ones-matmul
            sqaf = sb.tile([128, NA], F32, tag="sqaf")
            nc.scalar.activation(sqaf[:], esh_sb[:], AFT.Square)
            sqa_row = sb.tile([1, NA], F32, tag="sqa_row")
            sq_row = sb.tile([1, BPAD], F32, tag="sq_row")
            eb = sb.tile([128, NCLASS, TPC, 128], F16, tag="eb")
            sqf = sb.tile([128, NCLASS, TPC, 128], F32, tag="sqf")
            with tc.tile_pool(name="ps_sq", bufs=2, space=bass.MemorySpace.PSUM) as ps_sq:
                for h, w in ((0, 512), (512, 512), (1024, 256)):
                    pt = ps_sq.tile([1, 512], F32, tag="pt")
                    nc.tensor.matmul(pt[0:1, 0:w], ones[:], sqaf[:, h:h + w],
                                     start=True, stop=True)
                    nc.scalar.copy(sqa_row[0:1, h:h + w], pt[0:1, 0:w])

                # full matrix: gath[(t d), (k q)] -> eb[d, k, t, q]
                # (core t's shard holds tile t of every class k)
                nc.sync.dma_start(
                    eb[:], gath[:].rearrange("(t d) (k q) -> d k t q", d=128, k=NCLASS)
                )
                nc.scalar.activation(sqf[:], eb[:], AFT.Square)
                for k in range(NCLASS):
                    for t0 in (0, 4):
                        pt = ps_sq.tile([1, 512], F32, tag="pt")
                        nc.tensor.matmul(pt[0:1, :], ones[:], sqf[:, k, t0:t0 + 4, :],
                                         start=True, stop=True)
                        nc.scalar.copy(
                            sq_row[0:1, k * SEG + t0 * 128: k * SEG + t0 * 128 + 512],
                            pt[0:1, :])

            # reshape the [1, N] rows to per-partition layout via a DRAM bounce
            scr_a = dram.tile([1, NA], F32, tag="scr_a")
            scr_j = dram.tile([1, BPAD], F32, tag="scr_j")
            nc.sync.dma_start(scr_a[:], sqa_row[:])
            nc.sync.dma_start(scr_j[:], sq_row[:])
            sqa_pk = sb.tile([128, NCLASS], F32, tag="sqa_pk")
            nc.sync.dma_start(sqa_pk[:], scr_a[:].rearrange("a (k q) -> (a q) k", q=128))
            sqv = sb.tile([128, NJT], F32, tag="sqv")
            nc.sync.dma_start(sqv[:], scr_j[:].rearrange("a (t q) -> (a q) t", q=128))

            # ---- main sweep over 80 j-tiles, accumulating in 2*dot - sq_j space
            acc_hn = sb.tile([128, NA], F32, tag="acc_hn")
            acc_hp = sb.tile([128, NA], F32, tag="acc_hp")
            nc.vector.memset(acc_hn[:], -3.0e38)
            nc.vector.memset(acc_hp[:], 3.0e38)

            with tc.tile_pool(name="ps_g", bufs=2, space=bass.MemorySpace.PSUM) as ps_g:
                for t in range(NJT):
                    c, ri = t // TPC, t % TPC
                    nreal = min(max(int(R[c]) - ri * 128, 0), 128)
                    if nreal == 0:
                        # pure padding rows: can never win a hardest-neg/pos
                        continue
                    g = ps_g.tile([128, NA], F32, tag="g")
                    for h, w in ((0, 512), (512, 512), (1024, 256)):
                        nc.tensor.matmul(g[:, h:h + w], eb[:, c, ri, :],
                                         an2[:, h:h + w], start=True, stop=True)
                    sq_t = sqv[:, t:t + 1]
                    # hardest-negative: all anchor columns except own class c
                    if c > 0:
                        nc.vector.scalar_tensor_tensor(
                            acc_hn[:, 0:c * 128], g[:, 0:c * 128], sq_t,
                            acc_hn[:, 0:c * 128], op0=ALU.subtract, op1=ALU.max)
                    if c < NCLASS - 1:
                        nc.vector.scalar_tensor_tensor(
                            acc_hn[:, (c + 1) * 128:NA], g[:, (c + 1) * 128:NA], sq_t,
                            acc_hn[:, (c + 1) * 128:NA], op0=ALU.subtract, op1=ALU.max)
                    # hardest-positive: own-class columns, real j rows only
                    sl = slice(c * 128, (c + 1) * 128)
                    nc.vector.scalar_tensor_tensor(
                        acc_hp[0:nreal, sl], g[0:nreal, sl], sqv[0:nreal, t:t + 1],
                        acc_hp[0:nreal, sl], op0=ALU.subtract, op1=ALU.min)

            # ---- fold on chip: transpose-reduce over j-partials, loss math,
            # and a final partition sum down to [1, 1]
            ident = sb.tile([128, 128], F32, tag="ident")
            masks.make_identity(nc, ident[:])
            hn_t = sb.tile([128, NCLASS], F32, tag="hn_t")
            hp_t = sb.tile([128, NCLASS], F32, tag="hp_t")
            with tc.tile_pool(name="ps_f", bufs=2, space=bass.MemorySpace.PSUM) as ps_f:
                for b in range(NCLASS):
                    pn = ps_f.tile([128, 128], F32, tag="pn")
                    nc.tensor.transpose(pn[:], acc_hn[:, b * 128:(b + 1) * 128], ident[:])
                    nc.vector.reduce_max(hn_t[:, b:b + 1], pn[:], axis=mybir.AxisListType.X)
                    pp = ps_f.tile([128, 128], F32, tag="pp")
                    nc.tensor.transpose(pp[:], acc_hp[:, b * 128:(b + 1) * 128], ident[:])
                    nc.vector.tensor_reduce(hp_t[:, b:b + 1], pp[:], op=ALU.min,
                                            axis=mybir.AxisListType.X)

                hn2 = sb.tile([128, NCLASS], F32, tag="hn2")
                nc.vector.tensor_tensor(hn2[:], sqa_pk[:], hn_t[:], op=ALU.subtract)
                nc.vector.tensor_scalar_max(hn2[:], hn2[:], 0.0)
                nc.scalar.sqrt(hn2[:], hn2[:])
                hp2 = sb.tile([128, NCLASS], F32, tag="hp2")
                nc.vector.tensor_tensor(hp2[:], sqa_pk[:], hp_t[:], op=ALU.subtract)
                nc.vector.tensor_scalar_max(hp2[:], hp2[:], 0.0)
                nc.scalar.sqrt(hp2[:], hp2[:])

                li = sb.tile([128, NCLASS], F32, tag="li")
                nc.vector.tensor_tensor(li[:], hp2[:], hn2[:], op=ALU.subtract)
                nc.vector.tensor_scalar(li[:], li[:], float(MARGIN), 0.0,
                                        op0=ALU.add, op1=ALU.max)
                nc.vector.tensor_tensor(li[:], li[:], amask_sb[:], op=ALU.mult)
                li1 = sb.tile([128, 1], F32, tag="li1")
                nc.vector.reduce_sum(li1[:], li[:], axis=mybir.AxisListType.X)
                pl = ps_f.tile([1, 1], F32, tag="pl")
                nc.tensor.matmul(pl[0:1, 0:1], li1[:], ones[:], start=True, stop=True)
                loss_sb = sb.tile([1, 1], F32, tag="loss_sb")
                nc.scalar.copy(loss_sb[:], pl[0:1, 0:1])
            nc.sync.dma_start(loss_d[:], loss_sb[:])
    nc.compile()
    return nc


def prepare(embeddings, labels):
    emb = np.ascontiguousarray(np.asarray(embeddings, dtype=np.float32))
    lab = np.asarray(labels).astype(np.int64).ravel()
    assert emb.shape == (B, D)
    order = np.argsort(lab, kind="stable")
    es = emb[order]
    counts = np.bincount(lab, minlength=NCLASS)
    assert counts.max() <= SEG, counts
    ep = np.zeros((BPAD, D), np.float32)
    ep[:, 0] = DUMMY_VAL
    ofs = np.concatenate([[0], np.cumsum(counts)])
    for c in range(NCLASS):
        ep[c * SEG: c * SEG + counts[c]] = es[ofs[c]:ofs[c + 1]]
    eph = ep.astype(np.float16)
    q = np.arange(128)
    in_maps = []
    for i in range(NCORES):
        rows = np.concatenate(
            [eph[k * SEG + i * 128: k * SEG + (i + 1) * 128] for k in range(NCLASS)], 0
        )
        esh = np.ascontiguousarray(rows.T)                       # [128, 1280] f16
        amask = (i * 128 + q[:, None] < counts[None, :]).astype(np.float32)
        in_maps.append({"esh": esh, "amask": amask})
    return in_maps, counts


def combine(results, counts=None, in_maps=None):
    total = 0.0
    for i in range(NCORES):
        total += float(np.asarray(results[i]["loss"], np.float32)[0, 0])
    return np.asarray(total / B, dtype=np.float32)


def kernel(embeddings, labels, _trace=False, _tmpdir=None):
    in_maps, counts = prepare(embeddings, labels)
    nc = build_nc(list(counts))
    res = run_bass_kernel_spmd(
        nc, in_maps, list(range(NCORES)), trace=_trace, tmpdir=_tmpdir
    )
    out = combine(res.results)
    if _trace:
        return out, res
    return out


# revision 6
# speedup vs baseline: 17.3039x; 1.0471x over previous
"""BatchHardTripletLoss on 8 TRN2 NeuronCores (Bass/Tile).

The warm-path cost of this problem is host<->device traffic over the axon
tunnel (~60 MB/s up, ~30 MB/s down), not on-chip compute, so the kernel is
built around moving as few bytes as possible:

  - Host: sort rows by label, pad every class segment to SEG=1024 rows with
    far-away dummy rows (first coord DUMMY_VAL -> sq ~ 1e6, so dummies never
    win a hardest-negative).  Core i uploads ONLY its anchor shard: tile i of
    every class, transposed, in fp16 ([128, 1280] = 327 KB), plus a 5 KB
    anchor-validity mask.  Total upload ~2.7 MB instead of ~105 MB.
  - On chip: an AllGather over NeuronLink reassembles the full padded
    embedding matrix (the 8 anchor shards tile it exactly).  Squared norms
    are computed on chip (square + ones-matmul partition reduction), the
    distance-matrix sweep accumulates hardest-pos/neg in "2*dot - sq_j"
    space via DVE scalar_tensor_tensor off PSUM, and a PE-transpose fold
    reduces everything to one f32 loss partial per core ([1,1] download).
  - Host folds 8 scalars.
"""

import numpy as np

import jax

# Cache compiled XLA executables on disk: run_bass_kernel_spmd builds a fresh
# jax.jit closure per call, so without this every warm call pays a ~165 ms
# recompile before dispatch.
try:
    jax.config.update("jax_compilation_cache_dir", "/tmp/jax_comp_cache")
    jax.config.update("jax_persistent_cache_min_compile_time_secs", 0.0)
    jax.config.update("jax_persistent_cache_min_entry_size_bytes", 0)
except Exception:
    pass

import concourse.bass as bass
import concourse.bacc as bacc
import concourse.tile as tile
from concourse import masks, mybir
from concourse.bass_utils import run_bass_kernel_spmd

B, D, NCLASS = 8192, 128, 10
SEG = 1024                 # padded rows per class
TPC = SEG // 128           # 128-row tiles per class = 8
NCORES = 8
BPAD = NCLASS * SEG        # 10240
NJT = BPAD // 128          # 80 j-tiles
NA = NCLASS * 128          # anchors per core = 1280
F32 = mybir.dt.float32
F16 = mybir.dt.float16
AFT = mybir.ActivationFunctionType
ALU = mybir.AluOpType
MARGIN = 1.0
DUMMY_VAL = 1000.0


def build_nc(R, gather_addr_space="Shared"):
    """R: real row count per class (0 <= R[k] <= SEG)."""
    nc = bacc.Bacc()
    esh_d = nc.dram_tensor("esh", [D, NA], F16, kind="ExternalInput")
    amask_d = nc.dram_tensor("amask", [128, NCLASS], F32, kind="ExternalInput")
    loss_d = nc.dram_tensor("loss", [1, 1], F32, kind="ExternalOutput")

    with tile.TileContext(nc) as tc:
        with (
            tc.tile_pool(name="sb", bufs=1) as sb,
            tc.tile_pool(name="dram", bufs=1, space="DRAM") as dram,
        ):
            # ---- kick off the AllGather first: cores exchange anchor shards
            # so each one can rebuild the full [128, 10240] embedding matrix.
            bounce = dram.tile([D, NA], F16, tag="bounce")
            gath = dram.tile([NCORES * D, NA], F16, tag="gath",
                             addr_space=gather_addr_space)
            nc.gpsimd.dma_start(bounce[:], esh_d[:])
            nc.gpsimd.collective_compute(
                "AllGather",
                ALU.bypass,
                replica_groups=[list(range(NCORES))],
                ins=[bounce.opt()],
                outs=[gath.opt()],
            )

            # ---- local anchor-side prep (overlaps the collective)
            esh_sb = sb.tile([D, NA], F16, tag="esh_sb")
            nc.sync.dma_start(esh_sb[:], esh_d[:])
            amask_sb = sb.tile([128, NCLASS], F32, tag="amask_sb")
            nc.sync.dma_start(amask_sb[:], amask_d[:])

            an2 = sb.tile([D, NA], F16, tag="an2")
            nc.vector.tensor_scalar_mul(an2[:], esh_sb[:], 2.0)

            ones = sb.tile([128, 1], F32, tag="ones")
            nc.vector.memset(ones[:], 1.0)

            # squared norms: square, then partition-reduce via ones-matmul
            sqaf = sb.tile([128, NA], F32, tag="sqaf")
            nc.scalar.activation(sqaf[:], esh_sb[:], AFT.Square)
            sqa_row = sb.tile([1, NA], F32, tag="sqa_row")
            sq_row = sb.tile([1, BPAD], F32, tag="sq_row")
            eb = sb.tile([128, NCLASS, TPC, 128], F16, tag="eb")
            sqf = sb.tile([128, NCLASS, TPC, 128], F32, tag="sqf")
            with tc.tile_pool(name="ps_sq", bufs=2, space=bass.MemorySpace.PSUM) as ps_sq:
                for h, w in ((0, 512), (512, 512), (1024, 256)):
                    pt = ps_sq.tile([1, 512], F32, tag="pt")
                    nc.tensor.matmul(pt[0:1, 0:w], ones[:], sqaf[:, h:h + w],
                                     start=True, stop=True)
                    nc.scalar.copy(sqa_row[0:1, h:h + w], pt[0:1, 0:w])

                # full matrix: gath[(t d), (k q)] -> eb[d, k, t, q]
                # (core t's shard holds tile t of every class k)
                nc.sync.dma_start(
                    eb[:], gath[:].rearrange("(t d) (k q) -> d k t q", d=128, k=NCLASS)
                )
                nc.scalar.activation(sqf[:], eb[:], AFT.Square)
                for k in range(NCLASS):
                    for t0 in (0, 4):
                        pt = ps_sq.tile([1, 512], F32, tag="pt")
                        nc.tensor.matmul(pt[0:1, :], ones[:], sqf[:, k, t0:t0 + 4, :],
                                         start=True, stop=True)
                        nc.scalar.copy(
                            sq_row[0:1, k * SEG + t0 * 128: k * SEG + t0 * 128 + 512],
                            pt[0:1, :])

            # reshape the [1, N] rows to per-partition layout via a DRAM bounce
            scr_a = dram.tile([1, NA], F32, tag="scr_a")
            scr_j = dram.tile([1, BPAD], F32, tag="scr_j")
            nc.sync.dma_start(scr_a[:], sqa_row[:])
            nc.sync.dma_start(scr_j[:], sq_row[:])
            sqa_pk = sb.tile([128, NCLASS], F32, tag="sqa_pk")
            nc.sync.dma_start(sqa_pk[:], scr_a[:].rearrange("a (k q) -> (a q) k", q=128))
            sqv = sb.tile([128, NJT], F32, tag="sqv")
            nc.sync.dma_start(sqv[:], scr_j[:].rearrange("a (t q) -> (a q) t", q=128))

            # ---- main sweep over 80 j-tiles, accumulating in 2*dot - sq_j space
            acc_hn = sb.tile([128, NA], F32, tag="acc_hn")
            acc_hp = sb.tile([128, NA], F32, tag="acc_hp")
            nc.vector.memset(acc_hn[:], -3.0e38)
            nc.vector.memset(acc_hp[:], 3.0e38)

            with tc.tile_pool(name="ps_g", bufs=2, space=bass.MemorySpace.PSUM) as ps_g:
                for t in range(NJT):
                    c, ri = t // TPC, t % TPC
                    nreal = min(max(int(R[c]) - ri * 128, 0), 128)
                    if nreal == 0:
                        # pure padding rows: can never win a hardest-neg/pos
                        continue
                    g = ps_g.tile([128, NA], F32, tag="g")
                    for h, w in ((0, 512), (512, 512), (1024, 256)):
                        nc.tensor.matmul(g[:, h:h + w], eb[:, c, ri, :],
                                         an2[:, h:h + w], start=True, stop=True)
                    sq_t = sqv[:, t:t + 1]
                    # hardest-negative: all anchor columns except own class c
                    if c > 0:
                        nc.vector.scalar_tensor_tensor(
                            acc_hn[:, 0:c * 128], g[:, 0:c * 128], sq_t,
                            acc_hn[:, 0:c * 128], op0=ALU.subtract, op1=ALU.max)
                    if c < NCLASS - 1:
                        nc.vector.scalar_tensor_tensor(
                            acc_hn[:, (c + 1) * 128:NA], g[:, (c + 1) * 128:NA], sq_t,
                            acc_hn[:, (c + 1) * 128:NA], op0=ALU.subtract, op1=ALU.max)
                    # hardest-positive: own-class columns, real j rows only
                    sl = slice(c * 128, (c + 1) * 128)
                    nc.vector.scalar_tensor_tensor(
                        acc_hp[0:nreal, sl], g[0:nreal, sl], sqv[0:nreal, t:t + 1],
                        acc_hp[0:nreal, sl], op0=ALU.subtract, op1=ALU.min)

            # ---- fold on chip: transpose-reduce over j-partials, loss math,
            # and a final partition sum down to [1, 1]
            ident = sb.tile([128, 128], F32, tag="ident")
            masks.make_identity(nc, ident[:])
            hn_t = sb.tile([128, NCLASS], F32, tag="hn_t")
            hp_t = sb.tile([128, NCLASS], F32, tag="hp_t")
            with tc.tile_pool(name="ps_f", bufs=2, space=bass.MemorySpace.PSUM) as ps_f:
                for b in range(NCLASS):
                    pn = ps_f.tile([128, 128], F32, tag="pn")
                    nc.tensor.transpose(pn[:], acc_hn[:, b * 128:(b + 1) * 128], ident[:])
                    nc.vector.reduce_max(hn_t[:, b:b + 1], pn[:], axis=mybir.AxisListType.X)
                    pp = ps_f.tile([128, 128], F32, tag="pp")
                    nc.tensor.transpose(pp[:], acc_hp[:, b * 128:(b + 1) * 128], ident[:])
                    nc.vector.tensor_reduce(hp_t[:, b:b + 1], pp[:], op=ALU.min,
                                            axis=mybir.AxisListType.X)

                hn2 = sb.tile([128, NCLASS], F32, tag="hn2")
                nc.vector.tensor_tensor(hn2[:], sqa_pk[:], hn_t[:], op=ALU.subtract)
                nc.vector.tensor_scalar_max(hn2[:], hn2[:], 0.0)
                nc.scalar.sqrt(hn2[:], hn2[:])
                hp2 = sb.tile([128, NCLASS], F32, tag="hp2")
                nc.vector.tensor_tensor(hp2[:], sqa_pk[:], hp_t[:], op=ALU.subtract)
                nc.vector.tensor_scalar_max(hp2[:], hp2[:], 0.0)
                nc.scalar.sqrt(hp2[:], hp2[:])

                li = sb.tile([128, NCLASS], F32, tag="li")
                nc.vector.tensor_tensor(li[:], hp2[:], hn2[:], op=ALU.subtract)
                nc.vector.tensor_scalar(li[:], li[:], float(MARGIN), 0.0,
                                        op0=ALU.add, op1=ALU.max)
                nc.vector.tensor_tensor(li[:], li[:], amask_sb[:], op=ALU.mult)
                li1 = sb.tile([128, 1], F32, tag="li1")
                nc.vector.reduce_sum(li1[:], li[:], axis=mybir.AxisListType.X)
                pl = ps_f.tile([1, 1], F32, tag="pl")
                nc.tensor.matmul(pl[0:1, 0:1], li1[:], ones[:], start=True, stop=True)
                loss_sb = sb.tile([1, 1], F32, tag="loss_sb")
                nc.scalar.copy(loss_sb[:], pl[0:1, 0:1])
            nc.sync.dma_start(loss_d[:], loss_sb[:])
    nc.compile()
    return nc


def prepare(embeddings, labels):
    emb = np.ascontiguousarray(np.asarray(embeddings, dtype=np.float32))
    lab = np.asarray(labels).astype(np.int64).ravel()
    assert emb.shape == (B, D)
    order = np.argsort(lab, kind="stable")
    es = emb[order]
    counts = np.bincount(lab, minlength=NCLASS)
    assert counts.max() <= SEG, counts
    ep = np.zeros((BPAD, D), np.float32)
    ep[:, 0] = DUMMY_VAL
    ofs = np.concatenate([[0], np.cumsum(counts)])
    for c in range(NCLASS):
        ep[c * SEG: c * SEG + counts[c]] = es[ofs[c]:ofs[c + 1]]
    eph = ep.astype(np.float16)
    q = np.arange(128)
    in_maps = []
    for i in range(NCORES):
        rows = np.concatenate(
            [eph[k * SEG + i * 128: k * SEG + (i + 1) * 128] for k in range(NCLASS)], 0
        )
        esh = np.ascontiguousarray(rows.T)                       # [128, 1280] f16
        amask = (i * 128 + q[:, None] < counts[None, :]).astype(np.float32)
        in_maps.append({"esh": esh, "amask": amask})
    return in_maps, counts


def combine(results, counts=None, in_maps=None):
    total = 0.0
    for i in range(NCORES):
        total += float(np.asarray(results[i]["loss"], np.float32)[0, 0])
    return np.asarray(total / B, dtype=np.float32)


def kernel(embeddings, labels, _trace=False, _tmpdir=None):
    in_maps, counts = prepare(embeddings, labels)
    nc = build_nc(list(counts))
    res = run_bass_kernel_spmd(
        nc, in_maps, list(range(NCORES)), trace=_trace, tmpdir=_tmpdir
    )
    out = combine(res.results)
    if _trace:
        return out, res
    return out


# revision 11
# speedup vs baseline: 17.5018x; 1.0114x over previous
"""BatchHardTripletLoss on 8 TRN2 NeuronCores (Bass/Tile).

The warm-path cost of this problem is host<->device traffic over the axon
tunnel (~60 MB/s up, ~30 MB/s down), not on-chip compute, so the kernel is
built around moving as few bytes as possible:

  - Host: sort rows by label, pad every class segment to SEG=1024 rows with
    far-away dummy rows (first coord DUMMY_VAL -> sq ~ 1e6, so dummies never
    win a hardest-negative).  Core i uploads ONLY its anchor shard: tile i of
    every class, transposed, in fp16 ([128, 1280] = 327 KB), plus a 5 KB
    anchor-validity mask.  Total upload ~2.7 MB instead of ~105 MB.
  - On chip: an AllGather over NeuronLink reassembles the full padded
    embedding matrix (the 8 anchor shards tile it exactly).  Squared norms
    are computed on chip (square + ones-matmul partition reduction), the
    distance-matrix sweep accumulates hardest-pos/neg in "2*dot - sq_j"
    space via DVE scalar_tensor_tensor off PSUM, and a PE-transpose fold
    reduces everything to one f32 loss partial per core ([1,1] download).
  - Host folds 8 scalars.
"""

import numpy as np

import jax

# Cache compiled XLA executables on disk: run_bass_kernel_spmd builds a fresh
# jax.jit closure per call, so without this every warm call pays a ~165 ms
# recompile before dispatch.
try:
    jax.config.update("jax_compilation_cache_dir", "/tmp/jax_comp_cache")
    jax.config.update("jax_persistent_cache_min_compile_time_secs", 0.0)
    jax.config.update("jax_persistent_cache_min_entry_size_bytes", 0)
except Exception:
    pass

import concourse.bass as bass
import concourse.bacc as bacc
import concourse.tile as tile
from concourse import masks, mybir
from concourse.bass_utils import run_bass_kernel_spmd

B, D, NCLASS = 8192, 128, 10
SEG = 1024                 # padded rows per class
TPC = SEG // 128           # 128-row tiles per class = 8
NCORES = 8
BPAD = NCLASS * SEG        # 10240
NJT = BPAD // 128          # 80 j-tiles
NA = NCLASS * 128          # anchors per core = 1280
F32 = mybir.dt.float32
F16 = mybir.dt.float16
AFT = mybir.ActivationFunctionType
ALU = mybir.AluOpType
MARGIN = 1.0
DUMMY_VAL = 1000.0


def build_nc(R, gather_addr_space="Shared"):
    """R: real row count per class (0 <= R[k] <= SEG)."""
    nc = bacc.Bacc()
    esh_d = nc.dram_tensor("esh", [D, NA], F16, kind="ExternalInput")
    amask_d = nc.dram_tensor("amask", [128, NCLASS], F32, kind="ExternalInput")
    loss_d = nc.dram_tensor("loss", [1, 1], F32, kind="ExternalOutput")

    with tile.TileContext(nc) as tc:
        with (
            tc.tile_pool(name="sb", bufs=1) as sb,
            tc.tile_pool(name="dram", bufs=1, space="DRAM") as dram,
        ):
            # ---- kick off the AllGather first: cores exchange anchor shards
            # so each one can rebuild the full [128, 10240] embedding matrix.
            bounce = dram.tile([D, NA], F16, tag="bounce")
            gath = dram.tile([NCORES * D, NA], F16, tag="gath",
                             addr_space=gather_addr_space)
            nc.gpsimd.dma_start(bounce[:], esh_d[:])
            nc.gpsimd.collective_compute(
                "AllGather",
                ALU.bypass,
                replica_groups=[list(range(NCORES))],
                ins=[bounce.opt()],
                outs=[gath.opt()],
            )

            # ---- local anchor-side prep (overlaps the collective)
            esh_sb = sb.tile([D, NA], F16, tag="esh_sb")
            nc.sync.dma_start(esh_sb[:], esh_d[:])
            amask_sb = sb.tile([128, NCLASS], F32, tag="amask_sb")
            nc.sync.dma_start(amask_sb[:], amask_d[:])

            an2 = sb.tile([D, NA], F16, tag="an2")
            nc.vector.tensor_scalar_mul(an2[:], esh_sb[:], 2.0)

            ones = sb.tile([128, 1], F32, tag="ones")
            nc.vector.memset(ones[:], 1.0)

            # squared norms: square, then partition-reduce via ones-matmul
            sqaf = sb.tile([128, NA], F32, tag="sqaf")
            nc.scalar.activation(sqaf[:], esh_sb[:], AFT.Square)
            sqa_row = sb.tile([1, NA], F32, tag="sqa_row")
            sq_row = sb.tile([1, BPAD], F32, tag="sq_row")
            eb = sb.tile([128, NCLASS, TPC, 128], F16, tag="eb")
            sqf = sb.tile([128, NCLASS, TPC, 128], F32, tag="sqf")
            with tc.tile_pool(name="ps_sq", bufs=2, space=bass.MemorySpace.PSUM) as ps_sq:
                for h, w in ((0, 512), (512, 512), (1024, 256)):
                    pt = ps_sq.tile([1, 512], F32, tag="pt")
                    nc.tensor.matmul(pt[0:1, 0:w], ones[:], sqaf[:, h:h + w],
                                     start=True, stop=True)
                    nc.scalar.copy(sqa_row[0:1, h:h + w], pt[0:1, 0:w])

                # full matrix: gath[(t d), (k q)] -> eb[d, k, t, q]
                # (core t's shard holds tile t of every class k)
                nc.sync.dma_start(
                    eb[:], gath[:].rearrange("(t d) (k q) -> d k t q", d=128, k=NCLASS)
                )
                nc.scalar.activation(sqf[:], eb[:], AFT.Square)
                for k in range(NCLASS):
                    for t0 in (0, 4):
                        pt = ps_sq.tile([1, 512], F32, tag="pt")
                        nc.tensor.matmul(pt[0:1, :], ones[:], sqf[:, k, t0:t0 + 4, :],
                                         start=True, stop=True)
                        nc.scalar.copy(
                            sq_row[0:1, k * SEG + t0 * 128: k * SEG + t0 * 128 + 512],
                            pt[0:1, :])

            # reshape the [1, N] rows to per-partition layout via a DRAM bounce
            scr_a = dram.tile([1, NA], F32, tag="scr_a")
            scr_j = dram.tile([1, BPAD], F32, tag="scr_j")
            nc.sync.dma_start(scr_a[:], sqa_row[:])
            nc.sync.dma_start(scr_j[:], sq_row[:])
            sqa_pk = sb.tile([128, NCLASS], F32, tag="sqa_pk")
            nc.sync.dma_start(sqa_pk[:], scr_a[:].rearrange("a (k q) -> (a q) k", q=128))
            sqv = sb.tile([128, NJT], F32, tag="sqv")
            nc.sync.dma_start(sqv[:], scr_j[:].rearrange("a (t q) -> (a q) t", q=128))

            # ---- main sweep over 80 j-tiles, accumulating in 2*dot - sq_j space
            acc_hn = sb.tile([128, NA], F32, tag="acc_hn")
            acc_hp = sb.tile([128, NA], F32, tag="acc_hp")
            nc.vector.memset(acc_hn[:], -3.0e38)
            nc.vector.memset(acc_hp[:], 3.0e38)

            with tc.tile_pool(name="ps_g", bufs=2, space=bass.MemorySpace.PSUM) as ps_g:
                for t in range(NJT):
                    c, ri = t // TPC, t % TPC
                    nreal = min(max(int(R[c]) - ri * 128, 0), 128)
                    if nreal == 0:
                        # pure padding rows: can never win a hardest-neg/pos
                        continue
                    g = ps_g.tile([128, NA], F32, tag="g")
                    for h, w in ((0, 512), (512, 512), (1024, 256)):
                        nc.tensor.matmul(g[:, h:h + w], eb[:, c, ri, :],
                                         an2[:, h:h + w], start=True, stop=True)
                    sq_t = sqv[:, t:t + 1]
                    # hardest-negative: all anchor columns except own class c
                    if c > 0:
                        nc.vector.scalar_tensor_tensor(
                            acc_hn[:, 0:c * 128], g[:, 0:c * 128], sq_t,
                            acc_hn[:, 0:c * 128], op0=ALU.subtract, op1=ALU.max)
                    if c < NCLASS - 1:
                        nc.vector.scalar_tensor_tensor(
                            acc_hn[:, (c + 1) * 128:NA], g[:, (c + 1) * 128:NA], sq_t,
                            acc_hn[:, (c + 1) * 128:NA], op0=ALU.subtract, op1=ALU.max)
                    # hardest-positive: own-class columns, real j rows only
                    sl = slice(c * 128, (c + 1) * 128)
                    nc.vector.scalar_tensor_tensor(
                        acc_hp[0:nreal, sl], g[0:nreal, sl], sqv[0:nreal, t:t + 1],
                        acc_hp[0:nreal, sl], op0=ALU.subtract, op1=ALU.min)

            # ---- fold on chip: transpose-reduce over j-partials, loss math,
            # and a final partition sum down to [1, 1]
            ident = sb.tile([128, 128], F32, tag="ident")
            masks.make_identity(nc, ident[:])
            hn_t = sb.tile([128, NCLASS], F32, tag="hn_t")
            hp_t = sb.tile([128, NCLASS], F32, tag="hp_t")
            with tc.tile_pool(name="ps_f", bufs=2, space=bass.MemorySpace.PSUM) as ps_f:
                for b in range(NCLASS):
                    pn = ps_f.tile([128, 128], F32, tag="pn")
                    nc.tensor.transpose(pn[:], acc_hn[:, b * 128:(b + 1) * 128], ident[:])
                    nc.vector.reduce_max(hn_t[:, b:b + 1], pn[:], axis=mybir.AxisListType.X)
                    pp = ps_f.tile([128, 128], F32, tag="pp")
                    nc.tensor.transpose(pp[:], acc_hp[:, b * 128:(b + 1) * 128], ident[:])
                    nc.vector.tensor_reduce(hp_t[:, b:b + 1], pp[:], op=ALU.min,
                                            axis=mybir.AxisListType.X)

                hn2 = sb.tile([128, NCLASS], F32, tag="hn2")
                nc.vector.tensor_tensor(hn2[:], sqa_pk[:], hn_t[:], op=ALU.subtract)
                nc.vector.tensor_scalar_max(hn2[:], hn2[:], 0.0)
                nc.scalar.sqrt(hn2[:], hn2[:])
                hp2 = sb.tile([128, NCLASS], F32, tag="hp2")
                nc.vector.tensor_tensor(hp2[:], sqa_pk[:], hp_t[:], op=ALU.subtract)
                nc.vector.tensor_scalar_max(hp2[:], hp2[:], 0.0)
                nc.scalar.sqrt(hp2[:], hp2[:])

                li = sb.tile([128, NCLASS], F32, tag="li")
                nc.vector.tensor_tensor(li[:], hp2[:], hn2[:], op=ALU.subtract)
                nc.vector.tensor_scalar(li[:], li[:], float(MARGIN), 0.0,
                                        op0=ALU.add, op1=ALU.max)
                nc.vector.tensor_tensor(li[:], li[:], amask_sb[:], op=ALU.mult)
                li1 = sb.tile([128, 1], F32, tag="li1")
                nc.vector.reduce_sum(li1[:], li[:], axis=mybir.AxisListType.X)
                pl = ps_f.tile([1, 1], F32, tag="pl")
                nc.tensor.matmul(pl[0:1, 0:1], li1[:], ones[:], start=True, stop=True)
                loss_sb = sb.tile([1, 1], F32, tag="loss_sb")
                nc.scalar.copy(loss_sb[:], pl[0:1, 0:1])
            nc.sync.dma_start(loss_d[:], loss_sb[:])
    nc.compile()
    return nc


def prepare(embeddings, labels):
    emb = np.ascontiguousarray(np.asarray(embeddings, dtype=np.float32))
    lab = np.asarray(labels).astype(np.int64).ravel()
    assert emb.shape == (B, D)
    order = np.argsort(lab, kind="stable")
    es = emb[order]
    counts = np.bincount(lab, minlength=NCLASS)
    assert counts.max() <= SEG, counts
    ep = np.zeros((BPAD, D), np.float32)
    ep[:, 0] = DUMMY_VAL
    ofs = np.concatenate([[0], np.cumsum(counts)])
    for c in range(NCLASS):
        ep[c * SEG: c * SEG + counts[c]] = es[ofs[c]:ofs[c + 1]]
    eph = ep.astype(np.float16)
    q = np.arange(128)
    in_maps = []
    for i in range(NCORES):
        rows = np.concatenate(
            [eph[k * SEG + i * 128: k * SEG + (i + 1) * 128] for k in range(NCLASS)], 0
        )
        esh = np.ascontiguousarray(rows.T)                       # [128, 1280] f16
        amask = (i * 128 + q[:, None] < counts[None, :]).astype(np.float32)
        in_maps.append({"esh": esh, "amask": amask})
    return in_maps, counts


def combine(results, counts=None, in_maps=None):
    total = 0.0
    for i in range(NCORES):
        total += float(np.asarray(results[i]["loss"], np.float32)[0, 0])
    return np.asarray(total / B, dtype=np.float32)


def kernel(embeddings, labels, _trace=False, _tmpdir=None):
    in_maps, counts = prepare(embeddings, labels)
    nc = build_nc(list(counts))
    res = run_bass_kernel_spmd(
        nc, in_maps, list(range(NCORES)), trace=_trace, tmpdir=_tmpdir
    )
    out = combine(res.results)
    if _trace:
        return out, res
    return out
